# revision 1
# baseline (speedup 1.0000x reference)
"""Causal self-attention kernel for Trainium2, 8-core SPMD.

Problem: B=4, L=2048, D=768, H=12 heads (hd=64); y = attn(x) @ w_proj + b_proj.

Sharding: core c handles batch b=c//2 and head-group g=c%2 (6 heads each).
Each core computes q/k/v and flash-style causal attention for its 6 heads
(transposed-scores layout, ones-augmented V for softmax denominators), then an
AllGather within each core pair exchanges the two head-group halves so every
core can run the full (K=768) output projection for its batch. Host keeps the
even core of each pair.

The q axis is processed in four 512-wide chunks; each chunk's AllGather and
projection overlap the next chunk's attention, so only the last chunk's
exchange is exposed.

All matmuls run in float32r (full PE rate; ~2e-4 rel err vs fp32).
"""

import numpy as np
import ml_dtypes

import concourse.bacc as bacc
import concourse.mybir as mybir
import concourse.tile as tile
from concourse.bass_utils import run_bass_kernel_spmd

F32 = mybir.dt.float32
F32R = mybir.dt.float32r
BF16 = mybir.dt.bfloat16

B, L, D = 4, 2048, 768
NHEAD = 12
HD = 64
NH = 6              # local heads per core
HDL = NH * HD       # 384: local head dims
NT = L // 128       # 16 L-tiles
KD = D // 128       # 6 D-tiles
NQ = 4              # q chunks
QW = L // NQ        # 512: q chunk width
NEG = -1.0e30

_CACHED_NC = None


def build_nc():
    nc = bacc.Bacc(None, num_devices=8, debug=False)

    x_d = nc.dram_tensor("x", [L, D], F32R, kind="ExternalInput")
    wqk_d = nc.dram_tensor("wqk", [D, 2 * HDL], F32R, kind="ExternalInput")
    wv_d = nc.dram_tensor("wv", [D, HDL], F32R, kind="ExternalInput")
    wp_d = nc.dram_tensor("wp", [D, D], F32R, kind="ExternalInput")
    bp_d = nc.dram_tensor("bp", [1, D], F32R, kind="ExternalInput")
    ones_row_d = nc.dram_tensor("ones_row", [1, 128], F32R, kind="ExternalInput")
    ident_d = nc.dram_tensor("ident", [128, 128], F32R, kind="ExternalInput")
    identb_d = nc.dram_tensor("identb", [128, 128], BF16, kind="ExternalInput")
    nmask_d = nc.dram_tensor("nmask", [128, 128], BF16, kind="ExternalInput")
    ones_d = nc.dram_tensor("ones", [128, NT * NH], BF16, kind="ExternalInput")
    out_d = nc.dram_tensor("out", [L, D], F32, kind="ExternalOutput")

    ag_in = [
        nc.dram_tensor(f"ag_in{qq}", [HDL, QW], F32R, kind="Internal")
        for qq in range(NQ)
    ]
    ag_out = [
        [
            nc.dram_tensor(f"ag_out{qq}_{half}", [D // 2, QW], F32R, kind="Internal")
            for half in range(2)
        ]
        for qq in range(NQ)
    ]

    with tile.TileContext(nc) as tc:
        with (
            tc.tile_pool(name="persist", bufs=1) as pers,
            tc.tile_pool(name="attn", bufs=1) as attn_pool,
            tc.tile_pool(name="work", bufs=2) as work,
            tc.tile_pool(name="psum", bufs=2, space="PSUM") as pp,
        ):
            # ---------------- Phase 0: constants + weights ----------------
            ident = pers.tile([128, 128], F32R)
            nc.sync.dma_start(ident[:], ident_d[:])
            nmask = pers.tile([128, 128], BF16)
            nc.sync.dma_start(nmask[:], nmask_d[:])
            identb = pers.tile([128, 128], BF16)
            nc.sync.dma_start(identb[:], identb_d[:])
            wqk = pers.tile([128, KD, 2 * HDL], F32R)
            nc.sync.dma_start(wqk[:], wqk_d[:].rearrange("(a p) n -> p a n", p=128))
            wv = pers.tile([128, KD, HDL], F32R)
            nc.sync.dma_start(wv[:], wv_d[:].rearrange("(a p) n -> p a n", p=128))

            # ---------------- Phase 1: xT = x.T via PE transposes ----------------
            xT_ctx = tc.tile_pool(name="xTpool", bufs=1)
            xT_pool = xT_ctx.__enter__()
            xT = [xT_pool.tile([128, L], F32R, name=f"xT{j}") for j in range(KD)]
            for i in range(NT):
                x_t = work.tile([128, D], F32R, tag="x_t", bufs=4)
                nc.sync.dma_start(x_t[:], x_d[128 * i : 128 * (i + 1), :])
                for j in range(KD):
                    tp = pp.tile([128, 128], F32R, tag="psA", bufs=2)
                    nc.tensor.transpose(tp[:], x_t[:, 128 * j : 128 * (j + 1)], ident[:])
                    nc.scalar.activation(
                        xT[j][:, 128 * i : 128 * (i + 1)],
                        tp[:],
                        mybir.ActivationFunctionType.Copy,
                    )

            # ---------------- Phase 2a: kqT = (x @ wqk).T ----------------
            # kqT[m] [128, L]; m=0..2: qT head pairs; m=3..5: kT head pairs
            kqT = [attn_pool.tile([128, L], BF16, name=f"kqT{m}") for m in range(6)]
            for m in range(6):
                for qc in range(L // 512):
                    pt = pp.tile([128, 512], F32, tag="psA", bufs=2)
                    for j in range(KD):
                        nc.tensor.matmul(
                            pt[:],
                            wqk[:, j, 128 * m : 128 * (m + 1)],
                            xT[j][:, 512 * qc : 512 * (qc + 1)],
                            start=(j == 0),
                            stop=(j == KD - 1),
                        )
                    nc.scalar.activation(
                        kqT[m][:, 512 * qc : 512 * (qc + 1)],
                        pt[:],
                        mybir.ActivationFunctionType.Copy,
                    )

            # ---------------- Phase 2b: v_aug [128, NT, NH*65] ----------------
            v_sb = attn_pool.tile([128, NT, NH * 65], BF16)
            nc.sync.dma_start(
                v_sb[:].rearrange("p a (h w) -> p a h w", h=NH)[:, :, :, 64:65],
                ones_d[:].rearrange("p (a h) -> p a h", a=NT).unsqueeze(-1),
            )
            for i in range(NT):
                pv = pp.tile([128, HDL], F32, tag="psA", bufs=2)
                for j in range(KD):
                    nc.tensor.matmul(
                        pv[:],
                        xT[j][:, 128 * i : 128 * (i + 1)],
                        wv[:, j, :],
                        start=(j == 0),
                        stop=(j == KD - 1),
                    )
                nc.scalar.activation(
                    v_sb[:].rearrange("p a (h w) -> p a h w", h=NH)[:, i, :, 0:64],
                    pv[:].rearrange("p (h w) -> p h w", h=NH),
                    mybir.ActivationFunctionType.Copy,
                )

            xT_ctx.__exit__(None, None, None)

            late_ctx = tc.tile_pool(name="late", bufs=1)
            late = late_ctx.__enter__()
            wp = late.tile([128, KD, D], F32R)
            nc.sync.dma_start(wp[:], wp_d[:].rearrange("(a p) n -> p a n", p=128))
            bp_row = late.tile([1, D], F32R)
            nc.sync.dma_start(bp_row[:], bp_d[:])
            ones_row = late.tile([1, 128], F32R)
            nc.sync.dma_start(ones_row[:], ones_row_d[:])

            # ------ Phases 3-5, chunked over q: attention -> AllGather -> proj.
            # proj(qq) is emitted after attention(qq+1) so the in-order PE
            # stream never waits on the collective: by the time PE reaches
            # proj(qq)'s matmuls, the AllGather has long completed under the
            # next quarter's attention.
            def emit_proj_tile(qq, aoT, i):
                # one 128-row output tile of the projection for chunk qq
                q0 = QW * qq
                osb = late.tile([128, D], F32, tag="osb", bufs=3)
                for nchunk in range(2):
                    ns = 384 * nchunk
                    po = pp.tile([128, 384], F32, tag="psA", bufs=2)
                    for j in range(KD):
                        nc.tensor.matmul(
                            po[:],
                            aoT[j][:, 128 * i : 128 * (i + 1)],
                            wp[:, j, ns : ns + 384],
                            start=(j == 0),
                            stop=False,
                        )
                    # bias: po += ones_row.T @ bp (outer product broadcast)
                    nc.tensor.matmul(
                        po[:],
                        ones_row[:],
                        bp_row[:, ns : ns + 384],
                        start=False,
                        stop=True,
                    )
                    nc.scalar.activation(
                        osb[:, ns : ns + 384],
                        po[:],
                        mybir.ActivationFunctionType.Copy,
                    )
                nc.sync.dma_start(
                    out_d[q0 + 128 * i : q0 + 128 * (i + 1), :], osb[:]
                )

            # proj tiles of chunk qq-1 are sprinkled between heads of chunk qq
            # as guaranteed-ready PE filler (keeps the PE dense and HAM warm).
            proj_queue = []

            def emit_attn_cc(qq):
                q0 = QW * qq
                q1 = q0 + QW
                aoT = []
                # software-pipeline: attn@v for tile i is emitted two tiles
                # behind scores+exp, so PE never waits on ACT's exp.
                deferred = []

                def flush_deferred():
                    h_, oa_, t_, qs_, W_, ex_ = deferred.pop(0)
                    nc.tensor.matmul(
                        oa_[:, qs_ - q0 :],
                        v_sb[:, t_, 65 * h_ : 65 * h_ + 65],
                        ex_[:, :W_],
                        start=(t_ == 0),
                        stop=(t_ == q1 // 128 - 1),
                    )
                    if t_ == q1 // 128 - 1:
                        # head done: stage out+denom in sbuf, normalize per head
                        h2 = h_
                        aou = late.tile([65, QW], F32, tag="aou", bufs=4)
                        nc.vector.tensor_copy(aou[:], oa_[:])
                        row0 = late.tile([1, QW], F32, tag="row0", bufs=3)
                        nc.sync.dma_start(row0[:], aou[64:65, :])
                        rdb = late.tile([64, QW], F32, tag="rdb", bufs=3)
                        nc.gpsimd.partition_broadcast(rdb[:], row0[:])
                        nc.vector.reciprocal(rdb[:], rdb[:])
                        ao = late.tile([64, QW], F32R, tag="rdb", bufs=3)
                        nc.gpsimd.tensor_mul(
                            out=ao[:], in0=aou[0:64, :], in1=rdb[:]
                        )
                        nc.sync.dma_start(
                            ag_in[qq][64 * h2 : 64 * (h2 + 1), :], ao[:]
                        )
                        if h2 in (2, NH - 1):
                            half = 0 if h2 == 2 else 1
                            nc.gpsimd.collective_compute(
                                "AllGather",
                                mybir.AluOpType.bypass,
                                replica_groups=[[0, 1], [2, 3], [4, 5], [6, 7]],
                                ins=[ag_in[qq][192 * half : 192 * (half + 1), :]],
                                outs=[ag_out[qq][half][:]],
                            )
                            for jj in range(3):
                                t_ = late.tile(
                                    [128, QW], F32R, tag="aoT", bufs=7,
                                    name=f"aoT{qq}_{half}_{jj}",
                                )
                                nc.sync.dma_start(
                                    t_[:],
                                    ag_out[qq][half][128 * jj : 128 * (jj + 1), :],
                                )
                                aoT.append(t_)

                for h in range(NH):
                    p, sub = h // 2, h % 2
                    qT_h = kqT[p]
                    kT_h = kqT[3 + p]
                    oa = pp.tile([65, QW], F32, tag="oa", bufs=2)
                    for t in range(q1 // 128):
                        qs = max(128 * t, q0)
                        W = q1 - qs
                        sp = pp.tile([128, QW], F32, tag="sp", bufs=4)
                        diag = 128 * t >= q0
                        nc.tensor.matmul(
                            sp[:, :W],
                            kT_h[64 * sub : 64 * sub + 64, 128 * t : 128 * (t + 1)],
                            qT_h[64 * sub : 64 * sub + 64, qs:q1],
                            start=True,
                            stop=not diag,
                            tile_position=(64 * sub, 0),
                        )
                        if diag:
                            # add causal mask into the diagonal block via PE:
                            # sp[:, :128] += ident.T @ nmask
                            nc.tensor.matmul(
                                sp[:, 0:128],
                                identb[:],
                                nmask[:],
                                start=False,
                                stop=True,
                            )
                        ex = work.tile([128, QW], BF16, tag="ex", bufs=6)
                        nc.scalar.activation(
                            ex[:, :W], sp[:, :W], mybir.ActivationFunctionType.Exp,
                            scale=0.125,
                        )
                        deferred.append((h, oa, t, qs, W, ex))
                        if len(deferred) > 2:
                            flush_deferred()
                    # PE filler between heads: one proj tile of the prev chunk
                    if h >= 2 and proj_queue:
                        emit_proj_tile(*proj_queue.pop(0))
                while deferred:
                    flush_deferred()
                proj_queue.extend((qq, aoT, i) for i in range(QW // 128))

            for qq in range(NQ):
                emit_attn_cc(qq)
            while proj_queue:
                emit_proj_tile(*proj_queue.pop(0))
            late_ctx.__exit__(None, None, None)

    nc.compile()
    return nc


def get_nc():
    global _CACHED_NC
    if _CACHED_NC is None:
        _CACHED_NC = build_nc()
    return _CACHED_NC


def make_in_maps(x, w_attn, w_proj, b_proj):
    x = np.asarray(x, dtype=np.float32)
    w_attn = np.asarray(w_attn, dtype=np.float32)
    w_proj = np.asarray(w_proj, dtype=np.float32)
    b_proj = np.asarray(b_proj, dtype=np.float32)

    ident = np.eye(128, dtype=np.float32)
    # nmask[kp, qf] = 0 if qf >= kp else -1e30 (strict upper triangle masked)
    nmask = np.where(
        np.arange(128)[None, :] >= np.arange(128)[:, None], 0.0, NEG
    ).astype(ml_dtypes.bfloat16)
    ones = np.ones((128, NT * NH), dtype=ml_dtypes.bfloat16)

    head_order = [0, 1, 2, 6, 7, 8, 3, 4, 5, 9, 10, 11]
    wp_perm = np.concatenate([w_proj[64 * h : 64 * (h + 1)] for h in head_order], axis=0)

    in_maps = []
    for c in range(8):
        b, g = c // 2, c % 2
        qcols = slice(HDL * g, HDL * (g + 1))
        kcols = slice(D + HDL * g, D + HDL * (g + 1))
        vcols = slice(2 * D + HDL * g, 2 * D + HDL * (g + 1))
        wqk = np.concatenate([w_attn[:, qcols], w_attn[:, kcols]], axis=1)
        in_maps.append(
            {
                "x": np.ascontiguousarray(x[b]),
                "wqk": np.ascontiguousarray(wqk),
                "wv": np.ascontiguousarray(w_attn[:, vcols]),
                "wp": np.ascontiguousarray(wp_perm),
                "bp": b_proj.reshape(1, D),
                "ones_row": np.ones((1, 128), dtype=np.float32),
                "ident": ident,
                "identb": ident.astype(ml_dtypes.bfloat16),
                "nmask": nmask,
                "ones": ones,
            }
        )
    return in_maps


def kernel(x, w_attn, w_proj, b_proj):
    nc = get_nc()
    in_maps = make_in_maps(x, w_attn, w_proj, b_proj)
    res = run_bass_kernel_spmd(nc, in_maps, core_ids=list(range(8)))
    out = np.stack([res.results[2 * b]["out"] for b in range(B)], axis=0)
    return out.astype(np.float32)



# revision 6
# speedup vs baseline: 4.0555x; 4.0555x over previous
"""Causal self-attention kernel for Trainium2, 8-core SPMD — transfer-optimized.

Problem: B=4, L=2048, D=768, H=12 heads (hd=64); y = attn(x) @ w_proj + b_proj.

The wall-clock per call is dominated by host<->device transfer over the axon
tunnel (~166MB/s up, ~30-50MB/s back, ~215ms fixed), so the design minimizes
bytes moved per call:

- All device inputs are bf16 (error gate is 2e-2; bf16 end-to-end lands ~5e-3).
- x is uploaded split: core c gets rows [1024*(c%2) : ...] of batch c//2; a
  device-side AllGather within each pair reconstructs the full batch.
- Weights are uploaded sharded 4-ways (each core gets 1/4 of the rows of its
  head-group's [q|k|v] columns and of its group's w_proj rows); device-side
  AllGathers over {0,2,4,6}/{1,3,5,7} reconstruct them.
- No attention AllGather: each core computes a row-parallel PARTIAL projection
  from its own 6 heads; a pairwise ReduceScatter(add) then leaves each core
  with its half of the batch's output rows (even core: first 256 of each
  512-row q-chunk; odd core: second 256). Output is bf16 [1024, 768] per core;
  the host interleaves and casts.
- Small constants (identity, causal mask, ones) are generated on device via
  memset/affine_select instead of being uploaded.

Per-call transfer: ~30MB up (incl. donated zero outputs) + 12.6MB back,
vs ~200MB for the naive replicated-f32 layout.

Core c handles batch b=c//2 and head-group g=c%2 (6 heads each), flash-style
causal attention in transposed-scores layout with ones-augmented V for the
softmax denominators. All matmuls are bf16 with f32 PSUM accumulation.
"""

import numpy as np
import ml_dtypes

import concourse.bacc as bacc
import concourse.mybir as mybir
import concourse.tile as tile
from concourse.bass_utils import run_bass_kernel_spmd

F32 = mybir.dt.float32
F32R = mybir.dt.float32r
BF16 = mybir.dt.bfloat16

B, L, D = 4, 2048, 768
NHEAD = 12
HD = 64
NH = 6              # local heads per core
HDL = NH * HD       # 384: local head dims
NT = L // 128       # 16 L-tiles
KD = D // 128       # 6 D-tiles
NQ = 4              # q chunks
QW = L // NQ        # 512: q chunk width
HL = L // 2         # 1024: x rows uploaded per core / output rows per core
RSW = QW // 2       # 256: output rows per core per q-chunk
NEG = -1.0e30

PAIRS = [[0, 1], [2, 3], [4, 5], [6, 7]]
QUADS = [[0, 2, 4, 6], [1, 3, 5, 7]]

_CACHED_NC = None


def build_nc():
    nc = bacc.Bacc(None, num_devices=8, debug=False)

    xh_d = nc.dram_tensor("xh", [HL, D], BF16, kind="ExternalInput")
    wqkv_d = nc.dram_tensor("wqkv", [D // 4, 3 * HDL], BF16, kind="ExternalInput")
    wp_d = nc.dram_tensor("wp", [HDL // 4, D], BF16, kind="ExternalInput")
    bp_d = nc.dram_tensor("bp", [1, D], BF16, kind="ExternalInput")
    out_d = nc.dram_tensor("out", [HL, D], BF16, kind="ExternalOutput")

    # collectives cannot touch IO tensors: stage through Internal DRAM
    xh_i = nc.dram_tensor("xh_i", [HL, D], BF16, kind="Internal")
    wqkv_i = nc.dram_tensor("wqkv_i", [D // 4, 3 * HDL], BF16, kind="Internal")
    wp_i = nc.dram_tensor("wp_i", [HDL // 4, D], BF16, kind="Internal")
    ro_d = nc.dram_tensor("ro", [HL, D], BF16, kind="Internal")

    xf_d = nc.dram_tensor("xf", [L, D], BF16, kind="Internal")
    wqkvf_d = nc.dram_tensor("wqkvf", [D, 3 * HDL], BF16, kind="Internal")
    wpf_d = nc.dram_tensor("wpf", [HDL, D], BF16, kind="Internal")
    pp_d = [
        nc.dram_tensor(f"pp{qq}", [QW, D], BF16, kind="Internal") for qq in range(NQ)
    ]

    with tile.TileContext(nc) as tc:
        with (
            tc.tile_pool(name="persist", bufs=1) as pers,
            tc.tile_pool(name="attn", bufs=1) as attn_pool,
            tc.tile_pool(name="work", bufs=2) as work,
            tc.tile_pool(name="psum", bufs=2, space="PSUM") as pp,
        ):
            # ---------------- Phase 0a: input AllGathers (DRAM->DRAM) --------
            nc.sync.dma_start(xh_i[:], xh_d[:])
            nc.sync.dma_start(wqkv_i[:], wqkv_d[:])
            nc.sync.dma_start(wp_i[:], wp_d[:])
            nc.gpsimd.collective_compute(
                "AllGather", mybir.AluOpType.bypass,
                replica_groups=PAIRS, ins=[xh_i[:]], outs=[xf_d[:]],
            )
            nc.gpsimd.collective_compute(
                "AllGather", mybir.AluOpType.bypass,
                replica_groups=QUADS, ins=[wqkv_i[:]], outs=[wqkvf_d[:]],
            )
            nc.gpsimd.collective_compute(
                "AllGather", mybir.AluOpType.bypass,
                replica_groups=QUADS, ins=[wp_i[:]], outs=[wpf_d[:]],
            )

            # ---------------- Phase 0b: constants generated on device --------
            tmp1 = pers.tile([128, 128], BF16)
            nc.vector.memset(tmp1[:], 1.0)
            identb = pers.tile([128, 128], BF16)
            # identb[p,f] = 1 if f==p else 0
            nc.gpsimd.affine_select(
                identb[:], tmp1[:], pattern=[[1, 128]],
                compare_op=mybir.AluOpType.is_equal, fill=0.0,
                channel_multiplier=-1,
            )
            tmp0 = pers.tile([128, 128], BF16)
            nc.vector.memset(tmp0[:], 0.0)
            nmask = pers.tile([128, 128], BF16)
            # nmask[kp,qf] = 0 if qf >= kp else -1e30
            nc.gpsimd.affine_select(
                nmask[:], tmp0[:], pattern=[[1, 128]],
                compare_op=mybir.AluOpType.is_ge, fill=NEG,
                channel_multiplier=-1,
            )
            ones_row = pers.tile([1, 128], BF16)
            nc.vector.memset(ones_row[:], 1.0)

            # ---------------- Phase 0c: weights to SBUF ----------------------
            wqk = pers.tile([128, KD, 2 * HDL], BF16)
            nc.sync.dma_start(
                wqk[:], wqkvf_d[:, 0 : 2 * HDL].rearrange("(a p) n -> p a n", p=128)
            )
            wv = pers.tile([128, KD, HDL], BF16)
            nc.sync.dma_start(
                wv[:],
                wqkvf_d[:, 2 * HDL : 3 * HDL].rearrange("(a p) n -> p a n", p=128),
            )
            wp = pers.tile([128, 3, D], BF16)
            nc.sync.dma_start(wp[:], wpf_d[:].rearrange("(a p) n -> p a n", p=128))
            bp_row = pers.tile([1, D], BF16)
            nc.sync.dma_start(bp_row[:], bp_d[:])

            # ---------------- Phase 1: xT = x.T via PE transposes ------------
            xT_ctx = tc.tile_pool(name="xTpool", bufs=1)
            xT_pool = xT_ctx.__enter__()
            xT = [xT_pool.tile([128, L], BF16, name=f"xT{j}") for j in range(KD)]
            for i in range(NT):
                x_t = work.tile([128, D], BF16, tag="x_t", bufs=4)
                nc.sync.dma_start(x_t[:], xf_d[128 * i : 128 * (i + 1), :])
                for j in range(KD):
                    tp = pp.tile([128, 128], BF16, tag="psA", bufs=2)
                    nc.tensor.transpose(
                        tp[:], x_t[:, 128 * j : 128 * (j + 1)], identb[:]
                    )
                    nc.scalar.activation(
                        xT[j][:, 128 * i : 128 * (i + 1)],
                        tp[:],
                        mybir.ActivationFunctionType.Copy,
                    )

            # ---------------- Phase 2a: kqT = (x @ wqk).T --------------------
            # kqT[m] [128, L]; m=0..2: qT head pairs; m=3..5: kT head pairs
            kqT = [attn_pool.tile([128, L], BF16, name=f"kqT{m}") for m in range(6)]
            for m in range(6):
                for qc in range(L // 512):
                    pt = pp.tile([128, 512], F32, tag="psA", bufs=2)
                    for j in range(KD):
                        nc.tensor.matmul(
                            pt[:],
                            wqk[:, j, 128 * m : 128 * (m + 1)],
                            xT[j][:, 512 * qc : 512 * (qc + 1)],
                            start=(j == 0),
                            stop=(j == KD - 1),
                        )
                    nc.scalar.activation(
                        kqT[m][:, 512 * qc : 512 * (qc + 1)],
                        pt[:],
                        mybir.ActivationFunctionType.Copy,
                    )

            # ---------------- Phase 2b: v_aug [128, NT, NH*65] ---------------
            v_sb = attn_pool.tile([128, NT, NH * 65], BF16)
            nc.vector.memset(
                v_sb[:].rearrange("p a (h w) -> p a h w", h=NH)[:, :, :, 64:65], 1.0
            )
            for i in range(NT):
                pv = pp.tile([128, HDL], F32, tag="psA", bufs=2)
                for j in range(KD):
                    nc.tensor.matmul(
                        pv[:],
                        xT[j][:, 128 * i : 128 * (i + 1)],
                        wv[:, j, :],
                        start=(j == 0),
                        stop=(j == KD - 1),
                    )
                nc.scalar.activation(
                    v_sb[:].rearrange("p a (h w) -> p a h w", h=NH)[:, i, :, 0:64],
                    pv[:].rearrange("p (h w) -> p h w", h=NH),
                    mybir.ActivationFunctionType.Copy,
                )

            xT_ctx.__exit__(None, None, None)

            late_ctx = tc.tile_pool(name="late", bufs=1)
            late = late_ctx.__enter__()

            # ------ Phases 3-5, chunked over q: attention -> partial proj ->
            # pairwise ReduceScatter into this core's half of the output rows.
            # proj(qq) tiles are emitted between heads of attention(qq+1) as
            # guaranteed-ready PE filler; the RS rides the gpsimd queue under
            # the next chunk's attention.
            def emit_proj_tile(qq, aoT, i):
                osb = late.tile([128, D], F32, tag="osb", bufs=3)
                for nchunk in range(2):
                    ns = 384 * nchunk
                    po = pp.tile([128, 384], F32, tag="psA", bufs=2)
                    for j in range(3):
                        nc.tensor.matmul(
                            po[:],
                            aoT[j][:, 128 * i : 128 * (i + 1)],
                            wp[:, j, ns : ns + 384],
                            start=(j == 0),
                            stop=False,
                        )
                    # half-bias (added once per pair member; RS sums to full)
                    nc.tensor.matmul(
                        po[:],
                        ones_row[:],
                        bp_row[:, ns : ns + 384],
                        start=False,
                        stop=True,
                    )
                    nc.scalar.activation(
                        osb[:, ns : ns + 384],
                        po[:],
                        mybir.ActivationFunctionType.Copy,
                    )
                osb_b = late.tile([128, D], BF16, tag="osbb", bufs=3)
                nc.vector.tensor_copy(osb_b[:], osb[:])
                nc.sync.dma_start(pp_d[qq][128 * i : 128 * (i + 1), :], osb_b[:])
                if i == QW // 128 - 1:
                    nc.gpsimd.collective_compute(
                        "ReduceScatter", mybir.AluOpType.add,
                        replica_groups=PAIRS,
                        ins=[pp_d[qq][:]],
                        outs=[ro_d[RSW * qq : RSW * (qq + 1), :]],
                    )
                    nc.sync.dma_start(
                        out_d[RSW * qq : RSW * (qq + 1), :],
                        ro_d[RSW * qq : RSW * (qq + 1), :],
                    )

            proj_queue = []

            def emit_attn_cc(qq):
                q0 = QW * qq
                q1 = q0 + QW
                # aoT[j] holds normalized attention output rows for local heads
                # (2j, 2j+1), this chunk's 512 q columns.
                aoT = [
                    late.tile([128, QW], BF16, tag="aoT", bufs=7, name=f"aoT{qq}_{j}")
                    for j in range(3)
                ]
                # software-pipeline: attn@v for tile i is emitted two tiles
                # behind scores+exp, so PE never waits on ACT's exp.
                deferred = []

                def flush_deferred():
                    h_, oa_, t_, qs_, W_, ex_ = deferred.pop(0)
                    nc.tensor.matmul(
                        oa_[:, qs_ - q0 :],
                        v_sb[:, t_, 65 * h_ : 65 * h_ + 65],
                        ex_[:, :W_],
                        start=(t_ == 0),
                        stop=(t_ == q1 // 128 - 1),
                    )
                    if t_ == q1 // 128 - 1:
                        # head done: stage out+denom in sbuf, normalize per head
                        aou = late.tile([65, QW], F32, tag="aou", bufs=4)
                        nc.vector.tensor_copy(aou[:], oa_[:])
                        row0 = late.tile([1, QW], F32, tag="row0", bufs=3)
                        nc.sync.dma_start(row0[:], aou[64:65, :])
                        rdb = late.tile([64, QW], F32, tag="rdb", bufs=3)
                        nc.gpsimd.partition_broadcast(rdb[:], row0[:])
                        nc.vector.reciprocal(rdb[:], rdb[:])
                        nc.gpsimd.tensor_mul(
                            out=aoT[h_ // 2][64 * (h_ % 2) : 64 * (h_ % 2) + 64, :],
                            in0=aou[0:64, :],
                            in1=rdb[:],
                        )

                for h in range(NH):
                    p, sub = h // 2, h % 2
                    qT_h = kqT[p]
                    kT_h = kqT[3 + p]
                    oa = pp.tile([65, QW], F32, tag="oa", bufs=2)
                    for t in range(q1 // 128):
                        qs = max(128 * t, q0)
                        W = q1 - qs
                        sp = pp.tile([128, QW], F32, tag="sp", bufs=4)
                        diag = 128 * t >= q0
                        nc.tensor.matmul(
                            sp[:, :W],
                            kT_h[64 * sub : 64 * sub + 64, 128 * t : 128 * (t + 1)],
                            qT_h[64 * sub : 64 * sub + 64, qs:q1],
                            start=True,
                            stop=not diag,
                            tile_position=(64 * sub, 0),
                        )
                        if diag:
                            # add causal mask into the diagonal block via PE:
                            # sp[:, :128] += ident.T @ nmask
                            nc.tensor.matmul(
                                sp[:, 0:128],
                                identb[:],
                                nmask[:],
                                start=False,
                                stop=True,
                            )
                        ex = work.tile([128, QW], BF16, tag="ex", bufs=6)
                        nc.scalar.activation(
                            ex[:, :W], sp[:, :W], mybir.ActivationFunctionType.Exp,
                            scale=0.125,
                        )
                        deferred.append((h, oa, t, qs, W, ex))
                        if len(deferred) > 2:
                            flush_deferred()
                    # PE filler between heads: one proj tile of the prev chunk
                    if h >= 2 and proj_queue:
                        emit_proj_tile(*proj_queue.pop(0))
                while deferred:
                    flush_deferred()
                proj_queue.extend((qq, aoT, i) for i in range(QW // 128))

            for qq in range(NQ):
                emit_attn_cc(qq)
            while proj_queue:
                emit_proj_tile(*proj_queue.pop(0))
            late_ctx.__exit__(None, None, None)

    nc.compile()
    return nc


def get_nc():
    global _CACHED_NC
    if _CACHED_NC is None:
        _CACHED_NC = build_nc()
    return _CACHED_NC


def make_in_maps(x, w_attn, w_proj, b_proj):
    bf16 = ml_dtypes.bfloat16
    x = np.asarray(x, dtype=np.float32)
    w_attn = np.asarray(w_attn, dtype=np.float32)
    w_proj = np.asarray(w_proj, dtype=np.float32)
    b_proj = np.asarray(b_proj, dtype=np.float32)

    xb = x.astype(bf16)  # [B, L, D]
    wq, wk, wv_ = (w_attn[:, k * D : (k + 1) * D] for k in range(3))
    wqkv_g = [
        np.concatenate(
            [
                wq[:, HDL * g : HDL * (g + 1)],
                wk[:, HDL * g : HDL * (g + 1)],
                wv_[:, HDL * g : HDL * (g + 1)],
            ],
            axis=1,
        ).astype(bf16)
        for g in range(2)
    ]  # [768, 1152] per group
    wp_g = [w_proj[HDL * g : HDL * (g + 1)].astype(bf16) for g in range(2)]
    bph = (b_proj * 0.5).reshape(1, D).astype(bf16)

    in_maps = []
    for c in range(8):
        b, g, r = c // 2, c % 2, c // 2
        in_maps.append(
            {
                "xh": xb[b, HL * g : HL * (g + 1)],
                "wqkv": wqkv_g[g][192 * r : 192 * (r + 1)],
                "wp": wp_g[g][96 * r : 96 * (r + 1)],
                "bp": bph,
            }
        )
    return in_maps


def kernel(x, w_attn, w_proj, b_proj):
    nc = get_nc()
    in_maps = make_in_maps(x, w_attn, w_proj, b_proj)
    res = run_bass_kernel_spmd(nc, in_maps, core_ids=list(range(8)))
    out = np.empty((B, L, D), np.float32)
    for b in range(B):
        ev = np.asarray(res.results[2 * b]["out"]).astype(np.float32)
        od = np.asarray(res.results[2 * b + 1]["out"]).astype(np.float32)
        for qq in range(NQ):
            out[b, QW * qq : QW * qq + RSW] = ev[RSW * qq : RSW * (qq + 1)]
            out[b, QW * qq + RSW : QW * (qq + 1)] = od[RSW * qq : RSW * (qq + 1)]
    return out
